# revision 1
# baseline (speedup 1.0000x reference)
"""Trainium2 Bass kernel for nn_LinearTransformerLayer_44495861187342.

Reference network (see problem): SGIRA block (self-attn MHA-16h -> LN ->
cross-attn -> LN -> gate blend -> FFN(gelu) -> LN) followed by a SAIGA block
(squeeze-excite MLP -> LN -> 4-head self-attn -> LN).  With the shipped
inputs gate == 1.0, so the cross-attention branch is algebraically dead and
memory_states is unused; a general path that includes it is kept for
gate != 1.

Sharding (8 NeuronCores): core c owns 512 rows = (batch c//2, half c%2) of
the [4, 1024, 1024] input.  Every row-local op (projections, FFN, layernorm,
softmax) shards perfectly.  Self-attention K/V are recomputed per core for
the full 1024-row batch (the input is replicated host-side, so no
communication), and the mid-network activation `se` is exchanged within each
core pair via one AllGather so the SAIGA attention can see the whole batch.

Layout: activations are kept feature-major in SBUF ([feat partitions, rows
free]) so every matmul contracts the partition dim against natural-layout
weights, layernorm/softmax feature reductions become cheap PE ones-matmuls,
and the device never transposes anything (the host pre-transposes x and
post-transposes the output).  Matmuls run with operands bitcast to float32r
(full-rate on the PE, fp32 storage); everything else is fp32.
"""

import contextlib

import numpy as np

import concourse.bass as bass
import concourse.mybir as mybir
import concourse.tile as tile
from concourse import bacc
from concourse import bass_utils

F32 = mybir.dt.float32
F32R = mybir.dt.float32r
AF = mybir.ActivationFunctionType
OP = mybir.AluOpType

D = 1024          # model dim
DFF = 4096        # ffn dim
D2 = 2048         # squeeze-excite dim
S = 1024          # full sequence rows per batch
R = 512           # rows owned per core
P = 128
C = D // P        # 8 feature chunks
CFF = DFF // P    # 32
C2 = D2 // P      # 16
H_SA = 16         # SGIRA heads (hd 64)
H_SG = 4          # SAIGA heads (hd 256)
N_CORES = 8
EPS = 1e-5

_CACHE = {}


def _mm(nc, out, lhsT, rhs, start, stop):
    nc.tensor.matmul(out, lhsT.bitcast(F32R), rhs.bitcast(F32R),
                     start=start, stop=stop)


def _build(include_cross: bool, with_vbias: bool):
    nc = bacc.Bacc("TRN2", target_bir_lowering=False, debug=False,
                   num_devices=N_CORES)

    def din(name, shape):
        return nc.dram_tensor(name, shape, F32, kind="ExternalInput")

    # feature-major inputs (host pre-transposed), own 512 rows first
    xT = din("xT", [D, S])
    wq = din("wq", [D, D]); wk = din("wk", [D, D]); wv = din("wv", [D, D])
    bq = din("bq", [P, C]); bk = din("bk", [P, C])
    wo = din("wo", [D, D]); bo = din("bo", [P, C])
    w1 = din("w1", [D, DFF]); b1 = din("b1", [P, CFF])
    w2 = din("w2", [DFF, D]); b2 = din("b2", [P, C])
    exw = din("exw", [D, D2]); exb = din("exb", [P, C2])
    sqw = din("sqw", [D2, D]); sqb = din("sqb", [P, C])
    qw = din("qw", [D, D]); qb = din("qb", [P, C])
    kw = din("kw", [D, D]); kb = din("kb", [P, C])
    vw = din("vw", [D, D])
    nsg = din("nsg", [P, C]); nsb = din("nsb", [P, C])
    nfg = din("nfg", [P, C]); nfb = din("nfb", [P, C])
    nrg = din("nrg", [P, C]); nrb = din("nrb", [P, C])
    if with_vbias:
        bv = din("bv", [1, D])
        vb = din("vb", [1, D])
    if include_cross:
        mT = din("mT", [D, S])
        cwq = din("cwq", [D, D]); cwk = din("cwk", [D, D]); cwv = din("cwv", [D, D])
        cbq = din("cbq", [P, C]); cbk = din("cbk", [P, C])
        cwo = din("cwo", [D, D]); cbo = din("cbo", [P, C])
        ncg = din("ncg", [P, C]); ncb = din("ncb", [P, C])
        gate_c = din("gate_c", [P, 1])      # broadcast gate
        gate_1mc = din("gate_1mc", [P, 1])  # broadcast (1 - gate)
        if with_vbias:
            cbv = din("cbv", [1, D])

    out_d = nc.dram_tensor("out", [D, R], F32, kind="ExternalOutput")

    with tile.TileContext(nc) as tc:
        with contextlib.ExitStack() as ctx, \
             nc.allow_low_precision("float32r tiles feeding the PE"):
            const = ctx.enter_context(tc.tile_pool(name="const", bufs=1))
            wpool = ctx.enter_context(tc.tile_pool(name="wpool", bufs=3))
            # single PSUM pool, exactly 8 banks across 4 tags (bufs per tag)
            psp = ctx.enter_context(tc.tile_pool(name="psp", bufs=1,
                                                 space="PSUM"))
            tmp = ctx.enter_context(tc.tile_pool(name="tmp", bufs=2))
            small = ctx.enter_context(tc.tile_pool(name="small", bufs=1))

            def ps_mm(width=R):
                return psp.tile([P, width], F32, tag="mm", bufs=2, name="psmm")

            ones_f = const.tile([P, P], F32, tag="ones_f")
            nc.vector.memset(ones_f[:], 1.0)
            ones_col = const.tile([P, 1], F32R, tag="ones_col")
            nc.scalar.copy(ones_col[:], ones_f[:, 0:1])
            ones_row = const.tile([1, P], F32R, tag="ones_row")
            nc.scalar.copy(ones_row[:], ones_f[0:1, :])
            eps_t = const.tile([1, 1], F32, tag="eps")
            nc.vector.memset(eps_t[:], EPS)

            def load_pc(dram, dt=F32):  # small per-partition tables
                t = const.tile(list(dram.shape), dt, tag=dram.name + "_sb")
                src_ap = dram.ap().bitcast(dt) if dt is F32R else dram.ap()
                nc.sync.dma_start(out=t[:], in_=src_ap)
                return t

            bq_s = load_pc(bq); bk_s = load_pc(bk)
            bo_s = load_pc(bo); b1_s = load_pc(b1); b2_s = load_pc(b2)
            exb_s = load_pc(exb); sqb_s = load_pc(sqb)
            qb_s = load_pc(qb); kb_s = load_pc(kb)
            nsg_s = load_pc(nsg); nsb_s = load_pc(nsb)
            nfg_s = load_pc(nfg); nfb_s = load_pc(nfb)
            nrg_s = load_pc(nrg); nrb_s = load_pc(nrb)
            bv_s = load_pc(bv, F32R) if with_vbias else None
            vb_s = load_pc(vb, F32R) if with_vbias else None
            if include_cross:
                cbq_s = load_pc(cbq); cbk_s = load_pc(cbk)
                cbo_s = load_pc(cbo)
                ncg_s = load_pc(ncg); ncb_s = load_pc(ncb)
                gc_s = load_pc(gate_c); g1_s = load_pc(gate_1mc)
                cbv_s = load_pc(cbv, F32R) if with_vbias else None

            def wstat_ap(w, oc, kcs):
                # [P, kcs, 128] stationary block: w[(kc p), oc*128 + m]
                return w.ap().rearrange("(k p) m -> p k m", p=P)[:, :, oc * P:(oc + 1) * P]

            def proj(out_t, out_c, in_t, in_c, w, bias_s, rows, func=AF.Identity):
                """out_t[:, oc, r] = func(sum_k w[k, oc*128+p] * in_t[k, r] + b)."""
                for oc in range(out_c):
                    wt = wpool.tile([P, in_c, P], F32R, tag="wstat",
                                    name=f"wst_{w.name}{oc}")
                    nc.sync.dma_start(out=wt[:],
                                      in_=wstat_ap(w, oc, in_c).bitcast(F32R))
                    for rh in range(rows // R):
                        ps = ps_mm()
                        for kc in range(in_c):
                            _mm(nc, ps[:], wt[:, kc, :],
                                in_t[:, kc, rh * R:(rh + 1) * R],
                                start=(kc == 0), stop=(kc == in_c - 1))
                        nc.scalar.activation(
                            out_t[:, oc, rh * R:(rh + 1) * R], ps[:],
                            func, bias=bias_s[:, oc:oc + 1])

            def vproj(copy_fn, src, w_v, vbias_s, wmpool):
                """Row-major V projection over all S rows in quarter blocks.

                copy_fn(rc, q, ps) stores the [P, 256] psum result for the
                256 output features of quarter q, key-row chunk rc."""
                for q in range(4):
                    wm = wmpool.tile([P, C, 256], F32R, tag="wmov",
                                     name=f"wm{q}")
                    nc.sync.dma_start(
                        out=wm[:],
                        in_=w_v.ap().rearrange("(k p) m -> p k m", p=P)
                        [:, :, q * 256:(q + 1) * 256].bitcast(F32R))
                    for rc in range(C):
                        ps = ps_mm(256)
                        for kc in range(C):
                            _mm(nc, ps[:], src[:, kc, rc * P:(rc + 1) * P],
                                wm[:, kc, :], start=(kc == 0),
                                stop=(kc == C - 1 and vbias_s is None))
                        if vbias_s is not None:
                            _mm(nc, ps[:], ones_row[:],
                                vbias_s[0:1, q * 256:(q + 1) * 256],
                                start=False, stop=True)
                        copy_fn(rc, q, ps)

            def layer_norm(a_t, n_c, g_s, b_s, out_t):
                """Row-wise LN over the (P * n_c) feature dim of a_t [P,n_c,R]."""
                inv_n = 1.0 / (n_c * P)
                ps_sum = psp.tile([1, R], F32, tag="score", bufs=2,
                                  name="lnsum")
                ps_sq = psp.tile([1, R], F32, tag="score", bufs=2,
                                 name="lnsumsq")
                for c in range(n_c):
                    sqc = tmp.tile([P, R], F32R, tag="lnsqc", name="lnsqc")
                    nc.vector.tensor_mul(sqc[:], a_t[:, c, :], a_t[:, c, :])
                    _mm(nc, ps_sum[:], ones_col[:], a_t[:, c, :],
                        start=(c == 0), stop=(c == n_c - 1))
                    _mm(nc, ps_sq[:], ones_col[:], sqc[:],
                        start=(c == 0), stop=(c == n_c - 1))
                mu = small.tile([1, R], F32R, tag="lnmu", name="lnmu")
                nc.scalar.activation(mu[:], ps_sum[:], AF.Copy, scale=inv_n)
                va = small.tile([1, R], F32, tag="lnva", name="lnva")
                nc.scalar.activation(va[:], ps_sq[:], AF.Copy, scale=inv_n)
                mu2 = small.tile([1, R], F32, tag="lnmu2", name="lnmu2")
                nc.vector.tensor_mul(mu2[:], mu[:], mu[:])
                nc.vector.tensor_sub(va[:], va[:], mu2[:])
                nc.scalar.activation(va[:], va[:], AF.Sqrt, bias=eps_t[:])
                rstd = small.tile([1, R], F32R, tag="lnrstd", name="lnrstd")
                nc.vector.reciprocal(rstd[:], va[:])
                nc.vector.tensor_mul(mu[:], mu[:], rstd[:])  # mu*rstd
                bca_ps = psp.tile([P, R], F32, tag="bcast", bufs=2,
                                  name="lnbca")
                _mm(nc, bca_ps[:], ones_row[:], rstd[:], start=True, stop=True)
                bcb_ps = psp.tile([P, R], F32, tag="bcast", bufs=2,
                                  name="lnbcb")
                _mm(nc, bcb_ps[:], ones_row[:], mu[:], start=True, stop=True)
                bca = tmp.tile([P, R], F32, tag="lnbcas", name="lnbcas")
                nc.scalar.copy(bca[:], bca_ps[:])
                bcb = tmp.tile([P, R], F32, tag="lnbcbs", name="lnbcbs")
                nc.scalar.copy(bcb[:], bcb_ps[:])
                for c in range(n_c):
                    nc.vector.tensor_mul(out_t[:, c, :], a_t[:, c, :], bca[:])
                    nc.vector.tensor_sub(out_t[:, c, :], out_t[:, c, :], bcb[:])
                    nc.vector.tensor_scalar(
                        out_t[:, c, :], out_t[:, c, :],
                        g_s[:, c:c + 1], b_s[:, c:c + 1], OP.mult, OP.add)

            def mha16(kv_src, q_src, w_q, w_k, w_v, bq_t, bk_t, bv_t, pools):
                """16-head attention; q over own R rows, k/v over S rows of
                kv_src.  Returns ctx feature-major [P, C, R]."""
                apool, vpool, kqpool, expool, wmpool = pools
                # V row-major with a ones column per head: [P, S/P, 16*65]
                v_sb = vpool.tile([P, C, H_SA * 65], F32R, tag="v_sa",
                                  name="v_sa")
                v4 = v_sb.rearrange("p r (h e) -> p r h e", e=65)
                nc.scalar.copy(
                    v4[:, :, :, 64],
                    ones_f[:, 0:H_SA * C].rearrange("p (r h) -> p r h", r=C))
                vproj(lambda rc, q, ps: nc.scalar.copy(
                          v4[:, rc, q * 4:(q + 1) * 4, 0:64],
                          ps.rearrange("p (h e) -> p h e", e=64)),
                      kv_src, w_v, bv_t, wmpool)
                ctx_t = apool.tile([P, C, R], F32R, tag="ctx_sa", name="ctx_sa")
                for oc in range(C):
                    # K chunk oc for all S rows; Q chunk oc for own R rows
                    wt = wpool.tile([P, C, P], F32R, tag="wstat",
                                    name=f"wstk{oc}")
                    nc.sync.dma_start(out=wt[:], in_=wstat_ap(w_k, oc, C).bitcast(F32R))
                    kf = kqpool.tile([P, S], F32R, tag="kf", name="kf")
                    for rh in range(2):
                        ps = ps_mm()
                        for kc in range(C):
                            _mm(nc, ps[:], wt[:, kc, :],
                                kv_src[:, kc, rh * R:(rh + 1) * R],
                                start=(kc == 0), stop=(kc == C - 1))
                        nc.scalar.activation(kf[:, rh * R:(rh + 1) * R], ps[:],
                                             AF.Identity, bias=bk_t[:, oc:oc + 1])
                    wtq = wpool.tile([P, C, P], F32R, tag="wstat",
                                     name=f"wstq{oc}")
                    nc.sync.dma_start(out=wtq[:], in_=wstat_ap(w_q, oc, C).bitcast(F32R))
                    qf = kqpool.tile([P, R], F32R, tag="qf", name="qf")
                    ps = ps_mm()
                    for kc in range(C):
                        _mm(nc, ps[:], wtq[:, kc, :], q_src[:, kc, 0:R],
                            start=(kc == 0), stop=(kc == C - 1))
                    nc.scalar.activation(qf[:], ps[:], AF.Identity,
                                         bias=bq_t[:, oc:oc + 1])
                    for hh in range(2):
                        h = oc * 2 + hh
                        po = hh * 64
                        ctx_ps = psp.tile([P, R], F32, tag="ctxps", bufs=2,
                                          name="ctxps")
                        for kc in range(C):
                            ps_s = psp.tile([P, R], F32, tag="score", bufs=2,
                                            name="score")
                            _mm(nc, ps_s[:], kf[po:po + 64, kc * P:(kc + 1) * P],
                                qf[po:po + 64, :], start=True, stop=True)
                            ex = expool.tile([P, R], F32R, tag="exp", name="ex")
                            nc.scalar.activation(ex[:], ps_s[:], AF.Exp,
                                                 scale=0.125)
                            _mm(nc, ctx_ps[:65, :], v4[:, kc, h, 0:65], ex[:],
                                start=(kc == 0), stop=(kc == C - 1))
                        rec = small.tile([1, R], F32R, tag="rec", bufs=2,
                                         name="rec")
                        nc.vector.reciprocal(rec[:], ctx_ps[64:65, :])
                        bc_ps = psp.tile([P, R], F32, tag="bcast", bufs=2,
                                         name="bcast")
                        _mm(nc, bc_ps[0:64, :], ones_row[0:1, 0:64], rec[:],
                            start=True, stop=True)
                        bc_sb = small.tile([64, R], F32, tag="bcsb", bufs=2,
                                           name="bcsb")
                        nc.scalar.copy(bc_sb[:], bc_ps[0:64, :])
                        nc.vector.tensor_mul(ctx_t[po:po + 64, oc, :],
                                             ctx_ps[0:64, :], bc_sb[:])
                return ctx_t

            # ---------------- phase 1: SGIRA self-attention ----------------
            ssp_cm = tc.tile_pool(name="ssp", bufs=1, side="right")
            ssp = ssp_cm.__enter__()
            ss = ssp.tile([P, C, R], F32R, tag="ss")
            with tc.tile_pool(name="p1", bufs=1) as p1, \
                 tc.tile_pool(name="p1kq", bufs=2) as p1kq, \
                 tc.tile_pool(name="p1ex", bufs=3) as p1ex, \
                 tc.tile_pool(name="p1wm", bufs=2) as p1wm:
                xT_s = p1.tile([P, C, S], F32R, tag="xT")
                nc.sync.dma_start(
                    out=xT_s[:],
                    in_=xT.ap().rearrange("(c p) r -> p c r", p=P)
                    .bitcast(F32R))
                ctx_sa = mha16(xT_s, xT_s, wq, wk, wv, bq_s, bk_s, bv_s,
                               (p1, p1, p1kq, p1ex, p1wm))
                # out-proj + residual + LN -> ss
                sa = p1.tile([P, C, R], F32R, tag="sa")
                proj(sa, C, ctx_sa, C, wo, bo_s, R)
                for c in range(C):
                    nc.vector.tensor_add(sa[:, c, :], sa[:, c, :],
                                         xT_s[:, c, 0:R])
                layer_norm(sa, C, nsg_s, nsb_s, ss)

            # ---------------- optional cross-attention (gate != 1) ---------
            if include_cross:
                fusedp_cm = tc.tile_pool(name="fusedp", bufs=1, side="right")
                fusedp = fusedp_cm.__enter__()
                fused = fusedp.tile([P, C, R], F32R, tag="fused")
                with tc.tile_pool(name="pc1", bufs=1) as pc1, \
                     tc.tile_pool(name="pc1kq", bufs=2) as pc1kq, \
                     tc.tile_pool(name="pc1ex", bufs=3) as pc1ex, \
                     tc.tile_pool(name="pc1wm", bufs=2) as pc1wm:
                    mT_s = pc1.tile([P, C, S], F32R, tag="mT")
                    nc.sync.dma_start(
                        out=mT_s[:],
                        in_=mT.ap().rearrange("(c p) r -> p c r", p=P)
                        .bitcast(F32R))
                    ctx_ca = mha16(mT_s, ss, cwq, cwk, cwv, cbq_s, cbk_s,
                                   cbv_s, (pc1, pc1, pc1kq, pc1ex, pc1wm))
                    ca = pc1.tile([P, C, R], F32R, tag="ca")
                    proj(ca, C, ctx_ca, C, cwo, cbo_s, R)
                    for c in range(C):
                        nc.vector.tensor_add(ca[:, c, :], ca[:, c, :],
                                             ss[:, c, :])
                    cs = pc1.tile([P, C, R], F32R, tag="cs")
                    layer_norm(ca, C, ncg_s, ncb_s, cs)
                    # fused = gate*ss + (1-gate)*cs
                    for c in range(C):
                        nc.vector.tensor_scalar(
                            fused[:, c, :], ss[:, c, :], gc_s[:, 0:1], None,
                            OP.mult)
                        nc.vector.tensor_scalar(
                            cs[:, c, :], cs[:, c, :], g1_s[:, 0:1], None,
                            OP.mult)
                        nc.vector.tensor_add(fused[:, c, :], fused[:, c, :],
                                             cs[:, c, :])
                ff_in = fused
            else:
                ff_in = ss

            # ---------------- phase 2: FFN ----------------
            hidp_cm = tc.tile_pool(name="hidp", bufs=1)
            hidp = hidp_cm.__enter__()
            hidden = hidp.tile([P, C, R], F32R, tag="hidden")
            with tc.tile_pool(name="p2", bufs=1) as p2:
                h1 = p2.tile([P, CFF, R], F32R, tag="h1")
                proj(h1, CFF, ff_in, C, w1, b1_s, R, func=AF.Gelu)
                ffo = p2.tile([P, C, R], F32R, tag="ffo")
                for oc in range(C):
                    wt2 = p2.tile([P, CFF, P], F32R, tag="wstat2", bufs=2,
                                  name=f"wst2_{oc}")
                    nc.sync.dma_start(out=wt2[:],
                                      in_=wstat_ap(w2, oc, CFF).bitcast(F32R))
                    ps = ps_mm()
                    for kc in range(CFF):
                        _mm(nc, ps[:], wt2[:, kc, :], h1[:, kc, :],
                            start=(kc == 0), stop=(kc == CFF - 1))
                    nc.scalar.activation(ffo[:, oc, :], ps[:], AF.Identity,
                                         bias=b2_s[:, oc:oc + 1])
                for c in range(C):
                    nc.vector.tensor_add(ffo[:, c, :], ffo[:, c, :],
                                         ff_in[:, c, :])
                layer_norm(ffo, C, nfg_s, nfb_s, hidden)
            # ss (or fused) no longer needed
            if include_cross:
                fusedp_cm.__exit__(None, None, None)
            ssp_cm.__exit__(None, None, None)

            # ---------------- phase 3: SAIGA squeeze-excite ----------------
            sep_cm = tc.tile_pool(name="sep", bufs=1, side="right")
            sep = sep_cm.__enter__()
            se_own = sep.tile([P, C, R], F32R, tag="se_own")
            with tc.tile_pool(name="p3", bufs=1) as p3:
                h2 = p3.tile([P, C2, R], F32R, tag="h2")
                proj(h2, C2, hidden, C, exw, exb_s, R, func=AF.Relu)
                sqo = p3.tile([P, C, R], F32R, tag="sqo")
                proj(sqo, C, h2, C2, sqw, sqb_s, R)
                for c in range(C):
                    nc.vector.tensor_add(sqo[:, c, :], sqo[:, c, :],
                                         hidden[:, c, :])
                layer_norm(sqo, C, nrg_s, nrb_s, se_own)
            hidp_cm.__exit__(None, None, None)

            # ------- phase 4: pairwise AllGather of se; phase 5: SAIGA -----
            with tc.tile_pool(name="p5", bufs=1) as p5, \
                 tc.tile_pool(name="p5kq", bufs=2) as p5kq, \
                 tc.tile_pool(name="p5ex", bufs=3) as p5ex, \
                 tc.tile_pool(name="p5wm", bufs=2) as p5wm, \
                 tc.tile_pool(name="dramp", bufs=1, space="DRAM") as dramp:
                in_b = dramp.tile([D, R], F32, tag="cc_in")
                gat = dramp.tile([2, D, R], F32, tag="cc_out")
                nc.gpsimd.dma_start(
                    out=in_b.rearrange("(c p) r -> p c r", p=P),
                    in_=se_own[:].bitcast(F32))
                nc.gpsimd.collective_compute(
                    "AllGather", OP.bypass,
                    replica_groups=[[0, 1], [2, 3], [4, 5], [6, 7]],
                    ins=[in_b.opt()], outs=[gat.opt()])
                se_full = p5.tile([P, C, S], F32R, tag="se_full")
                for r in range(2):
                    nc.sync.dma_start(
                        out=se_full[:, :, r * R:(r + 1) * R],
                        in_=gat[r].rearrange("(c p) r -> p c r", p=P)
                        .bitcast(F32R))

                # V2 row-major [P, S/P, 1024] (head hd=256)
                v2 = p5.tile([P, C, D], F32R, tag="v2")
                vproj(lambda rc, q, ps: nc.scalar.copy(
                          v2[:, rc, q * 256:(q + 1) * 256], ps[:]),
                      se_full, vw, vb_s, p5wm)

                ctx2 = p5.tile([P, C, R], F32R, tag="ctx2")
                for h in range(H_SG):
                    k2 = []
                    q2 = []
                    for i in range(2):
                        oc = 2 * h + i
                        wt = wpool.tile([P, C, P], F32R, tag="wstat",
                                        name=f"wstk2_{oc}")
                        nc.sync.dma_start(out=wt[:], in_=wstat_ap(kw, oc, C).bitcast(F32R))
                        kt = p5kq.tile([P, S], F32R, tag="k2", name=f"k2_{i}")
                        for rh in range(2):
                            ps = ps_mm()
                            for kc in range(C):
                                _mm(nc, ps[:], wt[:, kc, :],
                                    se_full[:, kc, rh * R:(rh + 1) * R],
                                    start=(kc == 0), stop=(kc == C - 1))
                            nc.scalar.activation(kt[:, rh * R:(rh + 1) * R],
                                                 ps[:], AF.Identity,
                                                 bias=kb_s[:, oc:oc + 1])
                        k2.append(kt)
                        wtq = wpool.tile([P, C, P], F32R, tag="wstat",
                                         name=f"wstq2_{oc}")
                        nc.sync.dma_start(out=wtq[:], in_=wstat_ap(qw, oc, C).bitcast(F32R))
                        qt = p5kq.tile([P, R], F32R, tag="q2", name=f"q2_{i}")
                        ps = ps_mm()
                        for kc in range(C):
                            _mm(nc, ps[:], wtq[:, kc, :], se_own[:, kc, :],
                                start=(kc == 0), stop=(kc == C - 1))
                        nc.scalar.activation(qt[:], ps[:], AF.Identity,
                                             bias=qb_s[:, oc:oc + 1])
                        q2.append(qt)
                    ctx_ps = [psp.tile([P, R], F32, tag="ctxps", bufs=2,
                                       name=f"ctxps{mh}")
                              for mh in range(2)]
                    sum_ps = psp.tile([1, R], F32, tag="bcast", bufs=2,
                                      name="asum")
                    for kc in range(C):
                        ps_s = psp.tile([P, R], F32, tag="score", bufs=2,
                                        name="score2")
                        _mm(nc, ps_s[:], k2[0][:, kc * P:(kc + 1) * P],
                            q2[0][:], start=True, stop=False)
                        _mm(nc, ps_s[:], k2[1][:, kc * P:(kc + 1) * P],
                            q2[1][:], start=False, stop=True)
                        ex = p5ex.tile([P, R], F32R, tag="exp", name="ex2")
                        nc.scalar.activation(ex[:], ps_s[:], AF.Exp,
                                             scale=0.0625)
                        _mm(nc, sum_ps[:], ones_col[:], ex[:],
                            start=(kc == 0), stop=(kc == C - 1))
                        for mh in range(2):
                            _mm(nc, ctx_ps[mh][:],
                                v2[:, kc, (h * 256 + mh * P):(h * 256 + (mh + 1) * P)],
                                ex[:], start=(kc == 0), stop=(kc == C - 1))
                    rec = small.tile([1, R], F32R, tag="rec", bufs=2,
                                     name="rec2")
                    nc.vector.reciprocal(rec[:], sum_ps[:])
                    bc_ps = psp.tile([P, R], F32, tag="bcast", bufs=2,
                                     name="bcast2")
                    _mm(nc, bc_ps[:], ones_row[:], rec[:], start=True,
                        stop=True)
                    bc_sb = tmp.tile([P, R], F32, tag="bcsb2", name="bcsb2")
                    nc.scalar.copy(bc_sb[:], bc_ps[:])
                    for mh in range(2):
                        nc.vector.tensor_mul(ctx2[:, 2 * h + mh, :],
                                             ctx_ps[mh][:], bc_sb[:])

                # ---------------- phase 6: final residual + LN -------------
                for c in range(C):
                    nc.vector.tensor_add(ctx2[:, c, :], ctx2[:, c, :],
                                         se_own[:, c, :])
                fin = p5.tile([P, C, R], F32, tag="fin")
                layer_norm(ctx2, C, nrg_s, nrb_s, fin)
                nc.sync.dma_start(
                    out=out_d.ap().rearrange("(c p) r -> p c r", p=P),
                    in_=fin[:])
            sep_cm.__exit__(None, None, None)

    nc.compile()
    return nc


def _pc(v):
    """[n*128] -> [128, n] per-partition layout."""
    v = np.asarray(v, np.float32)
    return np.ascontiguousarray(v.reshape(-1, P).T)


def kernel(**inputs):
    x = np.asarray(inputs["input_states"], np.float32)
    gate = float(np.asarray(inputs["gate"]).ravel()[0])
    include_cross = (gate != 1.0)

    bq, bk, bv = np.split(np.asarray(inputs["sa_in_b"], np.float32), 3)
    vb = np.asarray(inputs["v_b"], np.float32)
    cbv = (np.split(np.asarray(inputs["ca_in_b"], np.float32), 3)[2]
           if include_cross else np.zeros(1, np.float32))
    with_vbias = bool(np.any(bv) or np.any(vb) or np.any(cbv))

    key = (include_cross, with_vbias)
    if key not in _CACHE:
        _CACHE[key] = _build(include_cross, with_vbias)
    nc = _CACHE[key]

    wq, wk, wv = [np.ascontiguousarray(w) for w in
                  np.split(np.asarray(inputs["sa_in_w"], np.float32), 3, axis=1)]

    shared = {
        "wq": wq, "wk": wk, "wv": wv,
        "bq": _pc(bq), "bk": _pc(bk),
        "wo": np.ascontiguousarray(np.asarray(inputs["sa_out_w"], np.float32)),
        "bo": _pc(inputs["sa_out_b"]),
        "w1": np.ascontiguousarray(np.asarray(inputs["ffn_w1"], np.float32)),
        "b1": _pc(inputs["ffn_b1"]),
        "w2": np.ascontiguousarray(np.asarray(inputs["ffn_w2"], np.float32)),
        "b2": _pc(inputs["ffn_b2"]),
        "exw": np.ascontiguousarray(np.asarray(inputs["ex_w"], np.float32)),
        "exb": _pc(inputs["ex_b"]),
        "sqw": np.ascontiguousarray(np.asarray(inputs["sq_w"], np.float32)),
        "sqb": _pc(inputs["sq_b"]),
        "qw": np.ascontiguousarray(np.asarray(inputs["q_w"], np.float32)),
        "qb": _pc(inputs["q_b"]),
        "kw": np.ascontiguousarray(np.asarray(inputs["k_w"], np.float32)),
        "kb": _pc(inputs["k_b"]),
        "vw": np.ascontiguousarray(np.asarray(inputs["v_w"], np.float32)),
        "nsg": _pc(inputs["ns_g"]), "nsb": _pc(inputs["ns_b"]),
        "nfg": _pc(inputs["nf_g"]), "nfb": _pc(inputs["nf_b"]),
        "nrg": _pc(inputs["nrm_g"]), "nrb": _pc(inputs["nrm_b"]),
    }
    if with_vbias:
        shared["bv"] = np.ascontiguousarray(bv.reshape(1, D))
        shared["vb"] = np.ascontiguousarray(vb.reshape(1, D))
    if include_cross:
        m = np.asarray(inputs["memory_states"], np.float32)
        cwq, cwk, cwv = [np.ascontiguousarray(w) for w in
                         np.split(np.asarray(inputs["ca_in_w"], np.float32),
                                  3, axis=1)]
        cbq, cbk, cbv_ = np.split(np.asarray(inputs["ca_in_b"], np.float32), 3)
        shared.update({
            "cwq": cwq, "cwk": cwk, "cwv": cwv,
            "cbq": _pc(cbq), "cbk": _pc(cbk),
            "cwo": np.ascontiguousarray(
                np.asarray(inputs["ca_out_w"], np.float32)),
            "cbo": _pc(inputs["ca_out_b"]),
            "ncg": _pc(inputs["nc_g"]), "ncb": _pc(inputs["nc_b"]),
            "gate_c": np.full((P, 1), gate, np.float32),
            "gate_1mc": np.full((P, 1), 1.0 - gate, np.float32),
        })
        if with_vbias:
            shared["cbv"] = np.ascontiguousarray(cbv_.reshape(1, D))

    in_maps = []
    for c in range(N_CORES):
        b, hf = c // 2, c % 2
        xp = np.concatenate([x[b, hf * R:(hf + 1) * R],
                             x[b, (1 - hf) * R:(2 - hf) * R]], axis=0)
        m_in = dict(shared)
        m_in["xT"] = np.ascontiguousarray(xp.T)
        if include_cross:
            m_in["mT"] = np.ascontiguousarray(m[b].T)
        in_maps.append(m_in)

    res = bass_utils.run_bass_kernel_spmd(nc, in_maps,
                                          core_ids=list(range(N_CORES)))
    out = np.empty((4, S, D), np.float32)
    for c in range(N_CORES):
        b, hf = c // 2, c % 2
        out[b, hf * R:(hf + 1) * R, :] = res.results[c]["out"].T
    return out

